# revision 13
# baseline (speedup 1.0000x reference)
# Trainium2 Bass kernel for nn_NeuralAdaptiveNetwork.
#
# Model: emb matmul -> 2x (Linear + BatchNorm1d(channel=S) + ReLU) ->
# 2-layer LSTMCell scan over S=512 steps -> gated head -> (out, si_new).
#
# Strategy: data-parallel over batch (B=64 -> 8 per core). Phase A (everything
# before the LSTM) runs in transposed-activation layout (feature on partitions,
# tokens on free dim, tokens b-major) so BatchNorm stats come out of PSUM
# accumulation for free; the two BN stat vectors and the final |imp| mean are
# the only cross-core collectives (AllReduce over 8 cores). The LSTM runs with
# batch on partitions, activations as the stationary matmul operand (weights
# stream through the PE), two concurrent column-group matmul streams via
# tile_position, and the precomputed input gates injected into PSUM with an
# 8x8 identity matmul. All matmuls in bf16, c-state and stats in fp32.

import os

import ml_dtypes
import numpy as np

import concourse.bacc as bacc
import concourse.tile as tile
from concourse import mybir
from concourse.bass_utils import run_bass_kernel_spmd
from concourse.masks import make_identity

BF = mybir.dt.bfloat16
F32 = mybir.dt.float32
AF = mybir.ActivationFunctionType
ALU = mybir.AluOpType

NCORES = 8
B, S, INDIM, H, OUT = 64, 512, 1024, 512, 512
BL = B // NCORES          # local batch = 8
NT = BL * S               # local tokens = 4096, b-major: t = b*S + s
G = 4 * H                 # 2048 gate dim
EPS = 1e-5
ADAPTIVE_RATE = 0.01
N_STEPS = int(os.environ.get("KERNEL_N_STEPS", S))

bf16 = ml_dtypes.bfloat16


def _bcast(ap, reps):
    """(P, F) AP -> (P, reps, F) AP with broadcast (step 0) middle dim."""
    p, f = ap.shape
    r = ap.rearrange("p (o f) -> p o f", o=1)
    return r.broadcast_to((p, reps, f))


def build(nc, n_steps):
    # ---------------- I/O ----------------
    def inp(name, shape, dt=BF):
        return nc.dram_tensor(name, shape, dt, kind="ExternalInput")

    xT = inp("xT", [INDIM, NT])
    w_embT = inp("w_embT", [INDIM, H])
    b_embc = inp("b_embc", [128, 4], F32)
    w0T = inp("w0T", [H, H])
    b0c = inp("b0c", [128, 4], F32)
    w1T = inp("w1T", [H, H])
    b1c = inp("b1c", [128, 4], F32)
    wih0T = inp("wih0T", [H, G])
    bias0r = inp("bias0r", [1, G])
    whh0T = inp("whh0T", [H, G])
    w1catT = inp("w1catT", [2 * H, G])
    bias1r = inp("bias1r", [1, G])
    wcT = inp("wcT", [2 * H, H])
    bcr = inp("bcr", [1, H])
    ctxT = inp("ctxT", [H, BL])
    wi1T = inp("wi1T", [H, H])
    bi1r = inp("bi1r", [1, H])
    wi2T = inp("wi2T", [H, H])
    bi2r = inp("bi2r", [1, H])
    wfT = inp("wfT", [H, OUT])
    bfr = inp("bfr", [1, OUT])
    si_in = inp("si_in", [1, H], F32)

    out_d = nc.dram_tensor("out", [BL, OUT], F32, kind="ExternalOutput")
    si_out_d = nc.dram_tensor("si_new", [1, H], F32, kind="ExternalOutput")

    RG = [list(range(NCORES))]
    INV_N = 1.0 / (B * H)  # BN stat normalizer (global batch x hidden)
    CH = 512               # phase-A token chunk == one local batch element
    NCH = NT // CH         # 8 chunks

    with tile.TileContext(nc) as tc:
        with (
            tc.tile_pool(name="const", bufs=1) as const,
            tc.tile_pool(name="wpool", bufs=1) as wpool,
            tc.tile_pool(name="acts", bufs=1) as acts,
            tc.tile_pool(name="mm_in", bufs=2) as mm_in,
            tc.tile_pool(name="small", bufs=1) as small,
            tc.tile_pool(name="lstm", bufs=2) as lstm,
            tc.tile_pool(name="zps", bufs=2, space="PSUM") as zps,
            tc.tile_pool(name="tps", bufs=2, space="PSUM") as tps,
            tc.tile_pool(name="sps", bufs=2, space="PSUM") as sps,
            tc.tile_pool(name="dram", bufs=1, space="DRAM") as dram,
            tc.tile_pool(name="state", bufs=1) as state,
        ):
            # ---------------- constants ----------------
            i8 = const.tile([8, 8], BF, tag="i8")
            make_identity(nc, i8)
            ones_1x128 = const.tile([1, 128], BF, tag="o1x128")
            nc.vector.memset(ones_1x128, 1.0)
            ones_1x128f = const.tile([1, 128], F32, tag="o1x128f")
            nc.vector.memset(ones_1x128f, 1.0)
            ones_128x1 = const.tile([128, 1], BF, tag="o128x1")
            nc.vector.memset(ones_128x1, 1.0)
            ones_8x1 = const.tile([8, 1], BF, tag="o8x1")
            nc.vector.memset(ones_8x1, 1.0)
            eps_col = const.tile([128, 1], F32, tag="eps")
            nc.vector.memset(eps_col, EPS)

            # ---------------- weight loading helpers ----------------
            def wtile(dr, rows, cols, tag):
                # (rows, cols) DRAM -> (128, rows//128, cols) SBUF
                nk = rows // 128
                t = wpool.tile([128, nk, cols], BF, tag=tag)
                nc.sync.dma_start(
                    out=t, in_=dr.ap().rearrange("(k p) c -> p k c", p=128)
                )
                return t

            def rtile(dr, shape, tag, dt=BF):
                t = wpool.tile(shape, dt, tag=tag)
                nc.sync.dma_start(out=t, in_=dr.ap())
                return t

            # phase-A weights (tags shared with later weights of same shape)
            w_embT_s = wtile(w_embT, INDIM, H, "w8")       # later: wcT
            w0T_s = wtile(w0T, H, H, "w4a")                # later: wi1T, wfT
            w1T_s = wtile(w1T, H, H, "w4b")                # later: wi2T
            wih0T_s = wtile(wih0T, H, G, "wg16")           # later: whh0T
            bias0r_s = rtile(bias0r, [1, G], "biasg")      # later: bias1r
            b_embc_s = rtile(b_embc, [128, 4], "b_embc", F32)
            b0c_s = rtile(b0c, [128, 4], "b0c", F32)
            b1c_s = rtile(b1c, [128, 4], "b1c", F32)
            w1catT_s = wtile(w1catT, 2 * H, G, "w1catT")   # resident all run
            si_s = small.tile([1, H], F32, tag="si_s")
            nc.sync.dma_start(out=si_s, in_=si_in.ap())
            ctxT_s = state.tile([128, 4, BL], BF, tag="ctxT")
            nc.sync.dma_start(
                out=ctxT_s, in_=ctxT.ap().rearrange("(k p) b -> p k b", p=128)
            )

            xT_r = xT.ap().rearrange("(k p) t -> p k t", p=128)

            h2_dram = dram.tile([H, NT], BF, tag="h2d")  # pre-BN L0 out
            h2_dram_r = h2_dram.rearrange("(k p) t -> p k t", p=128)
            g0_dram = dram.tile([NT, G], BF, tag="g0")
            l1buf = acts.tile([128, 4, NT], BF, tag="l1buf")

            def stats_psums():
                ssum = sps.tile([1, S], F32, tag="stat")
                ssq = sps.tile([1, S], F32, tag="stat")
                return ssum, ssq

            def stat_mms(ssum, ssq, src, first, last):
                # src: (128, CH) bf16 slice; accumulate col-sums over partitions
                nc.tensor.matmul(ssum, ones_128x1, src, start=first, stop=last)
                sq = mm_in.tile([128, CH], BF, tag="c4")
                nc.vector.tensor_mul(out=sq, in0=src, in1=src)
                nc.tensor.matmul(ssq, ones_128x1, sq, start=first, stop=last)

            def copy_bias(dst, ps, bias_col, flip):
                # psum -> sbuf bf16 with per-partition bias add
                if flip:
                    nc.vector.tensor_scalar_add(out=dst, in0=ps, scalar1=bias_col)
                else:
                    nc.scalar.activation(out=dst, in_=ps, func=AF.Identity,
                                         bias=bias_col, scale=1.0)

            # ---- fused emb + L0 (+ BN0 stats), chunk = one batch element ----
            ssum0, ssq0 = stats_psums()
            for c in range(NCH):
                cr = slice(c * CH, (c + 1) * CH)
                xc = mm_in.tile([128, 8, CH], BF, tag="xc")
                nc.sync.dma_start(out=xc, in_=xT_r[:, :, cr])
                embc = mm_in.tile([128, 4, CH], BF, tag="c4")
                for m in range(4):
                    ps = zps.tile([128, CH], F32, tag="zp")
                    for k in range(8):
                        nc.tensor.matmul(
                            ps, w_embT_s[:, k, m * 128 : (m + 1) * 128],
                            xc[:, k, :], start=(k == 0), stop=(k == 7),
                        )
                    copy_bias(embc[:, m, :], ps, b_embc_s[:, m : m + 1], m % 2)
                l0c = mm_in.tile([128, 4, CH], BF, tag="c4")
                for m in range(4):
                    ps = zps.tile([128, CH], F32, tag="zp")
                    for k in range(4):
                        nc.tensor.matmul(
                            ps, w0T_s[:, k, m * 128 : (m + 1) * 128],
                            embc[:, k, :], start=(k == 0), stop=(k == 3),
                        )
                    copy_bias(l0c[:, m, :], ps, b0c_s[:, m : m + 1], (m + 1) % 2)
                    stat_mms(ssum0, ssq0, l0c[:, m, :],
                             first=(c == 0 and m == 0), last=(c == 7 and m == 3))
                nc.sync.dma_start(out=h2_dram_r[:, :, cr], in_=l0c)

            # ---- BN stat finalize: AllReduce + per-channel inv/minv ----
            def bn_finalize(ssum, ssq):
                stats = small.tile([1, 2 * S], F32, tag="stats")
                nc.vector.tensor_copy(out=stats[:, 0:S], in_=ssum)
                nc.vector.tensor_copy(out=stats[:, S : 2 * S], in_=ssq)
                cin = dram.tile([1, 2 * S], F32, tag="ccin")
                cout = dram.tile([1, 2 * S], F32, tag="ccout")
                nc.sync.dma_start(out=cin, in_=stats)
                nc.gpsimd.collective_compute(
                    "AllReduce", ALU.add, replica_groups=RG,
                    ins=[cin.opt()], outs=[cout.opt()],
                )
                nc.sync.dma_start(out=stats, in_=cout)
                mean = small.tile([1, S], F32, tag="sc_m")
                nc.vector.tensor_scalar_mul(out=mean, in0=stats[:, 0:S],
                                            scalar1=INV_N)
                var = small.tile([1, S], F32, tag="sc_v")
                nc.vector.tensor_scalar_mul(out=var, in0=stats[:, S : 2 * S],
                                            scalar1=INV_N)
                tmp = small.tile([1, S], F32, tag="sc_t")
                nc.vector.tensor_mul(out=tmp, in0=mean, in1=mean)
                nc.vector.tensor_sub(out=var, in0=var, in1=tmp)
                nc.scalar.activation(out=tmp, in_=var, func=AF.Sqrt,
                                     bias=eps_col[0:1, :])
                inv = small.tile([1, S], F32, tag="sc_i")
                nc.vector.reciprocal(out=inv, in_=tmp)
                nc.vector.tensor_mul(out=tmp, in0=mean, in1=inv)  # tmp = m*inv
                bps = zps.tile([128, 2 * S], F32, tag="zp")
                nc.tensor.matmul(bps[:, 0:S], ones_1x128f, inv,
                                 start=True, stop=True)
                nc.tensor.matmul(bps[:, S : 2 * S], ones_1x128f, tmp,
                                 start=True, stop=True)
                inv_b = small.tile([128, S], BF, tag="inv_b")
                nc.vector.tensor_copy(out=inv_b, in_=bps[:, 0:S])
                minv_b = small.tile([128, S], BF, tag="minv_b")
                nc.vector.tensor_copy(out=minv_b, in_=bps[:, S : 2 * S])
                return inv_b, minv_b

            inv0, minv0 = bn_finalize(ssum0, ssq0)
            inv0_bc, minv0_bc = _bcast(inv0, 4), _bcast(minv0, 4)

            # ---- L1 (+ BN1 stats): stream h2 back, normalize, matmul ----
            ssum1, ssq1 = stats_psums()
            for c in range(NCH):
                cr = slice(c * CH, (c + 1) * CH)
                h2c = mm_in.tile([128, 4, CH], BF, tag="c4")
                nc.sync.dma_start(out=h2c, in_=h2_dram_r[:, :, cr])
                nc.vector.tensor_mul(out=h2c, in0=h2c, in1=inv0_bc)
                nc.vector.tensor_sub(out=h2c, in0=h2c, in1=minv0_bc)
                nc.vector.tensor_scalar_max(out=h2c, in0=h2c, scalar1=0.0)
                for m in range(4):
                    ps = zps.tile([128, CH], F32, tag="zp")
                    for k in range(4):
                        nc.tensor.matmul(
                            ps, w1T_s[:, k, m * 128 : (m + 1) * 128],
                            h2c[:, k, :], start=(k == 0), stop=(k == 3),
                        )
                    copy_bias(l1buf[:, m, cr], ps, b1c_s[:, m : m + 1], m % 2)
                    stat_mms(ssum1, ssq1, l1buf[:, m, cr],
                             first=(c == 0 and m == 0), last=(c == 7 and m == 3))

            inv1, minv1 = bn_finalize(ssum1, ssq1)
            inv1_bc, minv1_bc = _bcast(inv1, 4), _bcast(minv1, 4)

            # normalize l1buf in place -> h3
            for c in range(NCH):
                d = l1buf[:, :, c * CH : (c + 1) * CH]
                nc.vector.tensor_mul(out=d, in0=d, in1=inv1_bc)
                nc.vector.tensor_sub(out=d, in0=d, in1=minv1_bc)
                nc.vector.tensor_scalar_max(out=d, in0=d, scalar1=0.0)

            # ---- g0pre = h3 @ Wih0s.T + bias0s -> DRAM (NT, G) bf16 ----
            for mt in range(NT // 128):
                for half in range(2):
                    hr = slice(half * 1024, (half + 1) * 1024)
                    ps = zps.tile([128, 1024], F32, tag="zp")
                    for q in range(2):
                        qr = slice(q * 512, (q + 1) * 512)
                        gqr = slice(half * 1024 + q * 512,
                                    half * 1024 + (q + 1) * 512)
                        for k in range(4):
                            nc.tensor.matmul(
                                ps[:, qr], l1buf[:, k, mt * 128 : (mt + 1) * 128],
                                wih0T_s[:, k, gqr], start=(k == 0), stop=False,
                            )
                        nc.tensor.matmul(ps[:, qr], ones_1x128, bias0r_s[:, gqr],
                                         start=False, stop=True)
                    gsb = mm_in.tile([128, 1024], BF, tag="gsb")
                    if (mt + half) % 2 == 0:
                        nc.vector.tensor_copy(out=gsb, in_=ps)
                    else:
                        nc.scalar.copy(out=gsb, in_=ps)
                    nc.sync.dma_start(
                        out=g0_dram[mt * 128 : (mt + 1) * 128, hr], in_=gsb
                    )

            # ---- load LSTM/head weights into freed phase-A slots ----
            whh0T_s = wtile(whh0T, H, G, "wg16")
            bias1r_s = rtile(bias1r, [1, G], "biasg")
            wcT_s = wtile(wcT, 2 * H, H, "w8")
            wi1T_s = wtile(wi1T, H, H, "w4a")
            wi2T_s = wtile(wi2T, H, H, "w4b")
            bcr_s = rtile(bcr, [1, H], "bcr")
            bi1r_s = rtile(bi1r, [1, H], "bi1r")
            bi2r_s = rtile(bi2r, [1, H], "bi2r")
            bfr_s = rtile(bfr, [1, OUT], "bfr")

            # ---------------- LSTM over n_steps ----------------
            h0T = state.tile([128, 4, BL], BF, tag="h0T")
            h1T = state.tile([128, 4, BL], BF, tag="h1T")
            nc.vector.memset(h0T, 0.0)
            nc.vector.memset(h1T, 0.0)
            c0 = state.tile([BL, H], F32, tag="c0")
            c1 = state.tile([BL, H], F32, tag="c1")
            nc.vector.memset(c0, 0.0)
            nc.vector.memset(c1, 0.0)
            h1_last = state.tile([BL, H], BF, tag="h1_last")
            nc.vector.memset(h1_last, 0.0)

            g0_steps = g0_dram.rearrange("(b s) g -> s b g", b=BL)
            GP = [0, 32, 64, 96]                 # col-group base partitions
            GS = [slice(j * 512, (j + 1) * 512) for j in range(4)]  # gate blocks

            def cell(z, hT, cc, h_out):
                # z: psum (128, 1024); rows 0:8 = [i|f], rows 64:72 = [2g|o]
                sz = lstm.tile([BL, G], BF, tag="sz")
                nc.scalar.activation(out=sz[:, 0:512], in_=z[0:BL, 0:512],
                                     func=AF.Sigmoid)
                nc.scalar.activation(out=sz[:, 512:1024],
                                     in_=z[32 : 32 + BL, 0:512],
                                     func=AF.Sigmoid)
                nc.scalar.activation(out=sz[:, 1024:1536],
                                     in_=z[64 : 64 + BL, 512:1024], func=AF.Tanh)
                nc.scalar.activation(out=sz[:, 1536:2048],
                                     in_=z[96 : 96 + BL, 512:1024],
                                     func=AF.Sigmoid)
                for jj in range(2):  # PE warm-keepers, fire after sigmoids
                    jp = sps.tile([1, 512], F32, tag="stat")
                    nc.tensor.matmul(jp, sz[:, jj : jj + 1],
                                     sz[:, jj * 512 : (jj + 1) * 512],
                                     start=True, stop=True)
                ig = lstm.tile([BL, H], BF, tag="ig")
                nc.vector.tensor_mul(out=ig, in0=sz[:, 0:H], in1=sz[:, 1024:1536])
                nc.vector.tensor_mul(out=cc, in0=sz[:, H : 2 * H], in1=cc)
                nc.vector.tensor_add(out=cc, in0=cc, in1=ig)
                tch = lstm.tile([BL, H], BF, tag="tch")
                nc.scalar.activation(out=tch, in_=cc, func=AF.Tanh)
                jp = sps.tile([1, 512], F32, tag="stat")
                nc.tensor.matmul(jp, tch[:, 0:1], tch, start=True, stop=True)
                nc.vector.tensor_mul(out=h_out, in0=sz[:, 1536:2048], in1=tch)
                # transpose h_out -> hT (128, 4, 8)
                tp = tps.tile([128, 4, BL], BF, tag="tp")
                for k in range(4):
                    nc.tensor.transpose(tp[:, k, :],
                                        h_out[:, k * 128 : (k + 1) * 128], i8)
                nc.vector.tensor_copy(out=hT, in_=tp)

            h0_out = lstm.tile([BL, H], BF, tag="h0o")
            for t in range(n_steps):
                g0t = lstm.tile([BL, G], BF, tag="g0t")
                nc.sync.dma_start(out=g0t, in_=g0_steps[t])
                # z0 = g0pre[t] + h0 @ Whh0s.T   (two column groups)
                z0 = zps.tile([128, 1024], F32, tag="zp")
                zb0 = [z0[p : p + BL, (j // 2) * 512 : (j // 2 + 1) * 512]
                       for j, p in enumerate(GP)]
                for k in range(4):
                    for j in range(4):
                        nc.tensor.matmul(
                            zb0[j], h0T[:, k, :], whh0T_s[:, k, GS[j]],
                            start=(k == 0), stop=False,
                            tile_position=(0, GP[j]),
                        )
                for j in range(4):
                    nc.tensor.matmul(zb0[j], i8, g0t[:, GS[j]],
                                     start=False, stop=True,
                                     tile_position=(0, GP[j]))
                cell(z0, h0T, c0, h0_out)

                # z1 = [h0', h1] @ W1cat.T + bias1
                z1 = zps.tile([128, 1024], F32, tag="zp")
                zb1 = [z1[p : p + BL, (j // 2) * 512 : (j // 2 + 1) * 512]
                       for j, p in enumerate(GP)]
                for k in range(4):
                    for j in range(4):
                        nc.tensor.matmul(
                            zb1[j], h0T[:, k, :], w1catT_s[:, k, GS[j]],
                            start=(k == 0), stop=False,
                            tile_position=(0, GP[j]),
                        )
                for k in range(4):
                    for j in range(4):
                        nc.tensor.matmul(
                            zb1[j], h1T[:, k, :], w1catT_s[:, 4 + k, GS[j]],
                            start=False, stop=False,
                            tile_position=(0, GP[j]),
                        )
                for j in range(4):
                    nc.tensor.matmul(zb1[j], ones_1x128[:, 0:BL],
                                     bias1r_s[:, GS[j]],
                                     start=False, stop=True,
                                     tile_position=(0, GP[j]))
                cell(z1, h1T, c1, h1_last)

            # ---------------- head ----------------
            def head_mm(lhs_tiles, wT_s_, bias_r, n):
                ps = zps.tile([128, n], F32, tag="zp")
                o = ps[0:BL, :]
                for k, lt in enumerate(lhs_tiles):
                    nc.tensor.matmul(o, lt, wT_s_[:, k, :],
                                     start=(k == 0), stop=False)
                nc.tensor.matmul(o, ones_1x128[:, 0:BL], bias_r,
                                 start=False, stop=True)
                return ps

            def layernorm_sigmoid(ps, dst):
                mv_s = small.tile([BL, 6], F32, tag="mv_s")
                nc.vector.bn_stats(out=mv_s, in_=ps[0:BL, :])
                mv = small.tile([BL, 2], F32, tag="mv")
                nc.vector.bn_aggr(out=mv, in_=mv_s)
                sd = small.tile([BL, 1], F32, tag="hsd")
                nc.scalar.activation(out=sd, in_=mv[:, 1:2], func=AF.Sqrt,
                                     bias=eps_col[0:BL, :])
                inv = small.tile([BL, 1], F32, tag="hinv")
                nc.vector.reciprocal(out=inv, in_=sd)
                tmp = small.tile([BL, H], F32, tag="lntmp")
                nc.vector.tensor_scalar(out=tmp, in0=ps[0:BL, :],
                                        scalar1=mv[:, 0:1], scalar2=inv,
                                        op0=ALU.subtract, op1=ALU.mult)
                nc.scalar.activation(out=dst, in_=tmp, func=AF.Sigmoid)

            def transpose_to(src_bf16, tag):
                tp = tps.tile([128, 4, BL], BF, tag="tp")
                for k in range(4):
                    nc.tensor.transpose(tp[:, k, :],
                                        src_bf16[:, k * 128 : (k + 1) * 128], i8)
                dst = small.tile([128, 4, BL], BF, tag=tag)
                nc.vector.tensor_copy(out=dst, in_=tp)
                return dst

            # comp gate
            cat_tiles = [h1T[:, k, :] for k in range(4)] + \
                        [ctxT_s[:, k, :] for k in range(4)]
            ps = head_mm(cat_tiles, wcT_s, bcr_s, H)
            comp = small.tile([BL, H], BF, tag="comp")
            layernorm_sigmoid(ps, comp)
            compressed = small.tile([BL, H], BF, tag="compr")
            nc.vector.tensor_mul(out=compressed, in0=h1_last, in1=comp)
            compT = transpose_to(compressed, "compT")

            # importance chain
            ps = head_mm([compT[:, k, :] for k in range(4)], wi1T_s, bi1r_s, H)
            i1 = small.tile([BL, H], BF, tag="i1")
            nc.scalar.activation(out=i1, in_=ps[0:BL, :], func=AF.Relu)
            i1T = transpose_to(i1, "i1T")
            ps = head_mm([i1T[:, k, :] for k in range(4)], wi2T_s, bi2r_s, H)
            imp = small.tile([BL, H], BF, tag="imp")
            layernorm_sigmoid(ps, imp)

            # si_new = si + rate * (mean|imp| - si)
            aimp = small.tile([BL, H], BF, tag="aimp")
            nc.scalar.activation(out=aimp, in_=imp, func=AF.Abs)
            sps_si = sps.tile([1, H], F32, tag="stat")
            nc.tensor.matmul(sps_si, ones_8x1, aimp, start=True, stop=True)
            sisum = small.tile([1, H], F32, tag="sisum")
            nc.vector.tensor_copy(out=sisum, in_=sps_si)
            cin = dram.tile([1, H], F32, tag="siin")
            cout = dram.tile([1, H], F32, tag="siout")
            nc.sync.dma_start(out=cin, in_=sisum)
            nc.gpsimd.collective_compute(
                "AllReduce", ALU.add, replica_groups=RG,
                ins=[cin.opt()], outs=[cout.opt()],
            )
            nc.sync.dma_start(out=sisum, in_=cout)
            nc.vector.tensor_scalar_mul(out=sisum, in0=sisum, scalar1=1.0 / B)
            nc.vector.tensor_sub(out=sisum, in0=sisum, in1=si_s)
            nc.vector.tensor_scalar_mul(out=sisum, in0=sisum,
                                        scalar1=ADAPTIVE_RATE)
            nc.vector.tensor_add(out=sisum, in0=sisum, in1=si_s)
            nc.sync.dma_start(out=si_out_d.ap(), in_=sisum)

            # final output
            wfT_s = wtile(wfT, H, OUT, "w4a")  # reuses wi1T slot
            ps = head_mm([compT[:, k, :] for k in range(4)], wfT_s, bfr_s, OUT)
            outf = small.tile([BL, OUT], F32, tag="outf")
            nc.vector.tensor_copy(out=outf, in_=ps[0:BL, :])
            nc.sync.dma_start(out=out_d.ap(), in_=outf)

    nc.compile()
    return nc


_CACHE = {}


def _get_compiled(n_steps):
    if n_steps not in _CACHE:
        nc = bacc.Bacc("TRN2", target_bir_lowering=False, num_devices=NCORES)
        _CACHE[n_steps] = build(nc, n_steps)
    return _CACHE[n_steps]


def _prep_inputs(inputs):
    f = lambda a: np.asarray(a, dtype=np.float32)
    bfc = lambda a: np.ascontiguousarray(a).astype(bf16)

    x = f(inputs["x"])
    ctx = f(inputs["context"])
    si = f(inputs["state_importance"]).reshape(1, H)

    wih0 = f(inputs["Wih0"])
    whh0 = f(inputs["Whh0"])
    bias0 = f(inputs["bih0"]) + f(inputs["bhh0"])
    wih1 = f(inputs["Wih1"])
    whh1 = f(inputs["Whh1"])
    bias1 = f(inputs["bih1"]) + f(inputs["bhh1"])

    colmaj = lambda b: np.ascontiguousarray(f(b).reshape(4, 128).T)

    shared = {
        "w_embT": bfc(f(inputs["W_emb"]).T),
        "b_embc": colmaj(inputs["b_emb"]),
        "w0T": bfc(f(inputs["W0"]).T),
        "b0c": colmaj(inputs["b0"]),
        "w1T": bfc(f(inputs["W1"]).T),
        "b1c": colmaj(inputs["b1"]),
        "wih0T": bfc(wih0.T),
        "bias0r": bfc(bias0.reshape(1, G)),
        "whh0T": bfc(whh0.T),
        "w1catT": bfc(np.concatenate([wih1.T, whh1.T], axis=0)),
        "bias1r": bfc(bias1.reshape(1, G)),
        "wcT": bfc(f(inputs["Wc"]).T),
        "bcr": bfc(f(inputs["bc"]).reshape(1, H)),
        "wi1T": bfc(f(inputs["Wi1"]).T),
        "bi1r": bfc(f(inputs["bi1"]).reshape(1, H)),
        "wi2T": bfc(f(inputs["Wi2"]).T),
        "bi2r": bfc(f(inputs["bi2"]).reshape(1, H)),
        "wfT": bfc(f(inputs["Wf"]).T),
        "bfr": bfc(f(inputs["bf"]).reshape(1, OUT)),
        "si_in": si,
    }

    in_maps = []
    for c in range(NCORES):
        xs = x[c * BL : (c + 1) * BL].reshape(NT, INDIM)
        m = dict(shared)
        m["xT"] = bfc(xs.T)
        m["ctxT"] = bfc(ctx[c * BL : (c + 1) * BL].T)
        in_maps.append(m)
    return in_maps


def kernel(trace=False, **inputs):
    nc = _get_compiled(N_STEPS)
    in_maps = _prep_inputs(inputs)
    res = run_bass_kernel_spmd(
        nc, in_maps, core_ids=list(range(NCORES)), trace=trace
    )
    out = np.concatenate([res.results[c]["out"] for c in range(NCORES)], axis=0)
    si_new = res.results[0]["si_new"]
    kernel.last_result = res
    return out, si_new


# revision 14
# speedup vs baseline: 1.0833x; 1.0833x over previous
# Trainium2 Bass kernel for nn_NeuralAdaptiveNetwork.
#
# Model: emb matmul -> 2x (Linear + BatchNorm1d(channel=S) + ReLU) ->
# 2-layer LSTMCell scan over S=512 steps -> gated head -> (out, si_new).
#
# Strategy: data-parallel over batch (B=64 -> 8 per core). Phase A (everything
# before the LSTM) runs in transposed-activation layout (feature on partitions,
# tokens on free dim, tokens b-major) so BatchNorm stats come out of PSUM
# accumulation for free; the two BN stat vectors and the final |imp| mean are
# the only cross-core collectives (AllReduce over 8 cores). The LSTM runs with
# batch on partitions, activations as the stationary matmul operand (weights
# stream through the PE), two concurrent column-group matmul streams via
# tile_position, and the precomputed input gates injected into PSUM with an
# 8x8 identity matmul. All matmuls in bf16, c-state and stats in fp32.

import os

import ml_dtypes
import numpy as np

import concourse.bacc as bacc
import concourse.tile as tile
from concourse import mybir
from concourse.bass_utils import run_bass_kernel_spmd
from concourse.masks import make_identity

BF = mybir.dt.bfloat16
F32 = mybir.dt.float32
AF = mybir.ActivationFunctionType
ALU = mybir.AluOpType

NCORES = 8
B, S, INDIM, H, OUT = 64, 512, 1024, 512, 512
BL = B // NCORES          # local batch = 8
NT = BL * S               # local tokens = 4096, b-major: t = b*S + s
G = 4 * H                 # 2048 gate dim
EPS = 1e-5
ADAPTIVE_RATE = 0.01
N_STEPS = int(os.environ.get("KERNEL_N_STEPS", S))

bf16 = ml_dtypes.bfloat16


def _bcast(ap, reps):
    """(P, F) AP -> (P, reps, F) AP with broadcast (step 0) middle dim."""
    p, f = ap.shape
    r = ap.rearrange("p (o f) -> p o f", o=1)
    return r.broadcast_to((p, reps, f))


def build(nc, n_steps):
    # ---------------- I/O ----------------
    def inp(name, shape, dt=BF):
        return nc.dram_tensor(name, shape, dt, kind="ExternalInput")

    xT = inp("xT", [INDIM, NT])
    w_embT = inp("w_embT", [INDIM, H])
    b_embc = inp("b_embc", [128, 4], F32)
    w0T = inp("w0T", [H, H])
    b0c = inp("b0c", [128, 4], F32)
    w1T = inp("w1T", [H, H])
    b1c = inp("b1c", [128, 4], F32)
    wih0T = inp("wih0T", [H, G])
    bias0r = inp("bias0r", [1, G])
    whh0T = inp("whh0T", [H, G])
    w1catT = inp("w1catT", [2 * H, G])
    bias1r = inp("bias1r", [1, G])
    wcT = inp("wcT", [2 * H, H])
    bcr = inp("bcr", [1, H])
    ctxT = inp("ctxT", [H, BL])
    wi1T = inp("wi1T", [H, H])
    bi1r = inp("bi1r", [1, H])
    wi2T = inp("wi2T", [H, H])
    bi2r = inp("bi2r", [1, H])
    wfT = inp("wfT", [H, OUT])
    bfr = inp("bfr", [1, OUT])
    si_in = inp("si_in", [1, H], F32)

    out_d = nc.dram_tensor("out", [BL, OUT], F32, kind="ExternalOutput")
    si_out_d = nc.dram_tensor("si_new", [1, H], F32, kind="ExternalOutput")

    RG = [list(range(NCORES))]
    INV_N = 1.0 / (B * H)  # BN stat normalizer (global batch x hidden)
    CH = 512               # phase-A token chunk == one local batch element
    NCH = NT // CH         # 8 chunks

    with tile.TileContext(nc) as tc:
        with (
            tc.tile_pool(name="const", bufs=1) as const,
            tc.tile_pool(name="wpool", bufs=1) as wpool,
            tc.tile_pool(name="acts", bufs=1) as acts,
            tc.tile_pool(name="mm_in", bufs=2) as mm_in,
            tc.tile_pool(name="small", bufs=1) as small,
            tc.tile_pool(name="lstm", bufs=2) as lstm,
            tc.tile_pool(name="zps", bufs=2, space="PSUM") as zps,
            tc.tile_pool(name="tps", bufs=2, space="PSUM") as tps,
            tc.tile_pool(name="sps", bufs=2, space="PSUM") as sps,
            tc.tile_pool(name="dram", bufs=1, space="DRAM") as dram,
            tc.tile_pool(name="state", bufs=1) as state,
        ):
            # ---------------- constants ----------------
            i8 = const.tile([8, 8], BF, tag="i8")
            make_identity(nc, i8)
            ones_1x128 = const.tile([1, 128], BF, tag="o1x128")
            nc.vector.memset(ones_1x128, 1.0)
            ones_1x128f = const.tile([1, 128], F32, tag="o1x128f")
            nc.vector.memset(ones_1x128f, 1.0)
            ones_128x1 = const.tile([128, 1], BF, tag="o128x1")
            nc.vector.memset(ones_128x1, 1.0)
            ones_8x1 = const.tile([8, 1], BF, tag="o8x1")
            nc.vector.memset(ones_8x1, 1.0)
            eps_col = const.tile([128, 1], F32, tag="eps")
            nc.vector.memset(eps_col, EPS)

            # ---------------- weight loading helpers ----------------
            def wtile(dr, rows, cols, tag):
                # (rows, cols) DRAM -> (128, rows//128, cols) SBUF
                nk = rows // 128
                t = wpool.tile([128, nk, cols], BF, tag=tag)
                nc.sync.dma_start(
                    out=t, in_=dr.ap().rearrange("(k p) c -> p k c", p=128)
                )
                return t

            def rtile(dr, shape, tag, dt=BF):
                t = wpool.tile(shape, dt, tag=tag)
                nc.sync.dma_start(out=t, in_=dr.ap())
                return t

            # phase-A weights (tags shared with later weights of same shape)
            w_embT_s = wtile(w_embT, INDIM, H, "w8")       # later: wcT
            w0T_s = wtile(w0T, H, H, "w4a")                # later: wi1T, wfT
            w1T_s = wtile(w1T, H, H, "w4b")                # later: wi2T
            wih0T_s = wtile(wih0T, H, G, "wg16")           # later: whh0T
            bias0r_s = rtile(bias0r, [1, G], "biasg")      # later: bias1r
            b_embc_s = rtile(b_embc, [128, 4], "b_embc", F32)
            b0c_s = rtile(b0c, [128, 4], "b0c", F32)
            b1c_s = rtile(b1c, [128, 4], "b1c", F32)
            w1catT_s = wtile(w1catT, 2 * H, G, "w1catT")   # resident all run
            si_s = small.tile([1, H], F32, tag="si_s")
            nc.sync.dma_start(out=si_s, in_=si_in.ap())
            ctxT_s = state.tile([128, 4, BL], BF, tag="ctxT")
            nc.sync.dma_start(
                out=ctxT_s, in_=ctxT.ap().rearrange("(k p) b -> p k b", p=128)
            )

            xT_r = xT.ap().rearrange("(k p) t -> p k t", p=128)

            h2_dram = dram.tile([H, NT], BF, tag="h2d")  # pre-BN L0 out
            h2_dram_r = h2_dram.rearrange("(k p) t -> p k t", p=128)
            g0_dram = dram.tile([NT, G], BF, tag="g0")
            l1buf = acts.tile([128, 4, NT], BF, tag="l1buf")

            def stats_psums():
                ssum = sps.tile([1, S], F32, tag="stat")
                ssq = sps.tile([1, S], F32, tag="stat")
                return ssum, ssq

            def stat_mms(ssum, ssq, src, first, last):
                # src: (128, CH) bf16 slice; accumulate col-sums over partitions
                nc.tensor.matmul(ssum, ones_128x1, src, start=first, stop=last)
                sq = mm_in.tile([128, CH], BF, tag="c4")
                nc.vector.tensor_mul(out=sq, in0=src, in1=src)
                nc.tensor.matmul(ssq, ones_128x1, sq, start=first, stop=last)

            def copy_bias(dst, ps, bias_col, flip):
                # psum -> sbuf bf16 with per-partition bias add
                if flip:
                    nc.vector.tensor_scalar_add(out=dst, in0=ps, scalar1=bias_col)
                else:
                    nc.scalar.activation(out=dst, in_=ps, func=AF.Identity,
                                         bias=bias_col, scale=1.0)

            # ---- fused emb + L0 (+ BN0 stats), chunk = one batch element ----
            ssum0, ssq0 = stats_psums()
            for c in range(NCH):
                cr = slice(c * CH, (c + 1) * CH)
                xc = mm_in.tile([128, 8, CH], BF, tag="xc")
                nc.sync.dma_start(out=xc, in_=xT_r[:, :, cr])
                embc = mm_in.tile([128, 4, CH], BF, tag="c4")
                for m in range(4):
                    ps = zps.tile([128, CH], F32, tag="zp")
                    for k in range(8):
                        nc.tensor.matmul(
                            ps, w_embT_s[:, k, m * 128 : (m + 1) * 128],
                            xc[:, k, :], start=(k == 0), stop=(k == 7),
                        )
                    copy_bias(embc[:, m, :], ps, b_embc_s[:, m : m + 1], m % 2)
                l0c = mm_in.tile([128, 4, CH], BF, tag="c4")
                for m in range(4):
                    ps = zps.tile([128, CH], F32, tag="zp")
                    for k in range(4):
                        nc.tensor.matmul(
                            ps, w0T_s[:, k, m * 128 : (m + 1) * 128],
                            embc[:, k, :], start=(k == 0), stop=(k == 3),
                        )
                    copy_bias(l0c[:, m, :], ps, b0c_s[:, m : m + 1], (m + 1) % 2)
                    stat_mms(ssum0, ssq0, l0c[:, m, :],
                             first=(c == 0 and m == 0), last=(c == 7 and m == 3))
                nc.sync.dma_start(out=h2_dram_r[:, :, cr], in_=l0c)

            # ---- BN stat finalize: AllReduce + per-channel inv/minv ----
            def bn_finalize(ssum, ssq):
                stats = small.tile([1, 2 * S], F32, tag="stats")
                nc.vector.tensor_copy(out=stats[:, 0:S], in_=ssum)
                nc.vector.tensor_copy(out=stats[:, S : 2 * S], in_=ssq)
                cin = dram.tile([1, 2 * S], F32, tag="ccin")
                cout = dram.tile([1, 2 * S], F32, tag="ccout")
                nc.sync.dma_start(out=cin, in_=stats)
                nc.gpsimd.collective_compute(
                    "AllReduce", ALU.add, replica_groups=RG,
                    ins=[cin.opt()], outs=[cout.opt()],
                )
                nc.sync.dma_start(out=stats, in_=cout)
                mean = small.tile([1, S], F32, tag="sc_m")
                nc.vector.tensor_scalar_mul(out=mean, in0=stats[:, 0:S],
                                            scalar1=INV_N)
                var = small.tile([1, S], F32, tag="sc_v")
                nc.vector.tensor_scalar_mul(out=var, in0=stats[:, S : 2 * S],
                                            scalar1=INV_N)
                tmp = small.tile([1, S], F32, tag="sc_t")
                nc.vector.tensor_mul(out=tmp, in0=mean, in1=mean)
                nc.vector.tensor_sub(out=var, in0=var, in1=tmp)
                nc.scalar.activation(out=tmp, in_=var, func=AF.Sqrt,
                                     bias=eps_col[0:1, :])
                inv = small.tile([1, S], F32, tag="sc_i")
                nc.vector.reciprocal(out=inv, in_=tmp)
                nc.vector.tensor_mul(out=tmp, in0=mean, in1=inv)  # tmp = m*inv
                bps = zps.tile([128, 2 * S], F32, tag="zp")
                nc.tensor.matmul(bps[:, 0:S], ones_1x128f, inv,
                                 start=True, stop=True)
                nc.tensor.matmul(bps[:, S : 2 * S], ones_1x128f, tmp,
                                 start=True, stop=True)
                inv_b = small.tile([128, S], BF, tag="inv_b")
                nc.vector.tensor_copy(out=inv_b, in_=bps[:, 0:S])
                minv_b = small.tile([128, S], BF, tag="minv_b")
                nc.vector.tensor_copy(out=minv_b, in_=bps[:, S : 2 * S])
                return inv_b, minv_b

            inv0, minv0 = bn_finalize(ssum0, ssq0)
            inv0_bc, minv0_bc = _bcast(inv0, 4), _bcast(minv0, 4)

            # ---- L1 (+ BN1 stats): stream h2 back, normalize, matmul ----
            ssum1, ssq1 = stats_psums()
            for c in range(NCH):
                cr = slice(c * CH, (c + 1) * CH)
                h2c = mm_in.tile([128, 4, CH], BF, tag="c4")
                nc.sync.dma_start(out=h2c, in_=h2_dram_r[:, :, cr])
                nc.vector.tensor_mul(out=h2c, in0=h2c, in1=inv0_bc)
                nc.vector.tensor_sub(out=h2c, in0=h2c, in1=minv0_bc)
                nc.vector.tensor_scalar_max(out=h2c, in0=h2c, scalar1=0.0)
                for m in range(4):
                    ps = zps.tile([128, CH], F32, tag="zp")
                    for k in range(4):
                        nc.tensor.matmul(
                            ps, w1T_s[:, k, m * 128 : (m + 1) * 128],
                            h2c[:, k, :], start=(k == 0), stop=(k == 3),
                        )
                    copy_bias(l1buf[:, m, cr], ps, b1c_s[:, m : m + 1], m % 2)
                    stat_mms(ssum1, ssq1, l1buf[:, m, cr],
                             first=(c == 0 and m == 0), last=(c == 7 and m == 3))

            inv1, minv1 = bn_finalize(ssum1, ssq1)
            inv1_bc, minv1_bc = _bcast(inv1, 4), _bcast(minv1, 4)

            # normalize l1buf in place -> h3
            for c in range(NCH):
                d = l1buf[:, :, c * CH : (c + 1) * CH]
                nc.vector.tensor_mul(out=d, in0=d, in1=inv1_bc)
                nc.vector.tensor_sub(out=d, in0=d, in1=minv1_bc)
                nc.vector.tensor_scalar_max(out=d, in0=d, scalar1=0.0)

            # ---- g0pre = h3 @ Wih0s.T + bias0s -> DRAM (NT, G) bf16 ----
            for mt in range(NT // 128):
                for half in range(2):
                    hr = slice(half * 1024, (half + 1) * 1024)
                    ps = zps.tile([128, 1024], F32, tag="zp")
                    for q in range(2):
                        qr = slice(q * 512, (q + 1) * 512)
                        gqr = slice(half * 1024 + q * 512,
                                    half * 1024 + (q + 1) * 512)
                        for k in range(4):
                            nc.tensor.matmul(
                                ps[:, qr], l1buf[:, k, mt * 128 : (mt + 1) * 128],
                                wih0T_s[:, k, gqr], start=(k == 0), stop=False,
                            )
                        nc.tensor.matmul(ps[:, qr], ones_1x128, bias0r_s[:, gqr],
                                         start=False, stop=True)
                    gsb = mm_in.tile([128, 1024], BF, tag="gsb")
                    if (mt + half) % 2 == 0:
                        nc.vector.tensor_copy(out=gsb, in_=ps)
                    else:
                        nc.scalar.copy(out=gsb, in_=ps)
                    nc.sync.dma_start(
                        out=g0_dram[mt * 128 : (mt + 1) * 128, hr], in_=gsb
                    )

            # ---- load LSTM/head weights into freed phase-A slots ----
            whh0T_s = wtile(whh0T, H, G, "wg16")
            bias1r_s = rtile(bias1r, [1, G], "biasg")
            wcT_s = wtile(wcT, 2 * H, H, "w8")
            wi1T_s = wtile(wi1T, H, H, "w4a")
            wi2T_s = wtile(wi2T, H, H, "w4b")
            bcr_s = rtile(bcr, [1, H], "bcr")
            bi1r_s = rtile(bi1r, [1, H], "bi1r")
            bi2r_s = rtile(bi2r, [1, H], "bi2r")
            bfr_s = rtile(bfr, [1, OUT], "bfr")

            # ---------------- LSTM over n_steps ----------------
            h0T = state.tile([128, 4, BL], BF, tag="h0T")
            h1T = state.tile([128, 4, BL], BF, tag="h1T")
            nc.vector.memset(h0T, 0.0)
            nc.vector.memset(h1T, 0.0)
            c0 = state.tile([BL, H], F32, tag="c0")
            c1 = state.tile([BL, H], F32, tag="c1")
            nc.vector.memset(c0, 0.0)
            nc.vector.memset(c1, 0.0)
            h1_last = state.tile([BL, H], BF, tag="h1_last")
            nc.vector.memset(h1_last, 0.0)

            g0_steps = g0_dram.rearrange("(b s) g -> s b g", b=BL)
            GP = [0, 32, 64, 96]                 # col-group base partitions
            GS = [slice(j * 512, (j + 1) * 512) for j in range(4)]  # gate blocks

            def cell(z, hT, cc, h_out):
                # z: psum (128, 1024); rows 0:8 = [i|f], rows 64:72 = [2g|o]
                sz = lstm.tile([BL, G], BF, tag="sz")
                nc.scalar.activation(out=sz[:, 0:512], in_=z[0:BL, 0:512],
                                     func=AF.Sigmoid)
                nc.scalar.activation(out=sz[:, 512:1024],
                                     in_=z[32 : 32 + BL, 0:512],
                                     func=AF.Sigmoid)
                nc.scalar.activation(out=sz[:, 1024:1536],
                                     in_=z[64 : 64 + BL, 512:1024], func=AF.Tanh)
                nc.scalar.activation(out=sz[:, 1536:2048],
                                     in_=z[96 : 96 + BL, 512:1024],
                                     func=AF.Sigmoid)
                jp = sps.tile([1, 512], F32, tag="stat")
                nc.tensor.matmul(jp, sz[:, 0:1], sz[:, 0:512],
                                 start=True, stop=True)
                ig = lstm.tile([BL, H], BF, tag="ig")
                nc.vector.tensor_mul(out=ig, in0=sz[:, 0:H], in1=sz[:, 1024:1536])
                nc.vector.tensor_mul(out=cc, in0=sz[:, H : 2 * H], in1=cc)
                nc.vector.tensor_add(out=cc, in0=cc, in1=ig)
                tch = lstm.tile([BL, H], BF, tag="tch")
                nc.scalar.activation(out=tch, in_=cc, func=AF.Tanh)
                nc.vector.tensor_mul(out=h_out, in0=sz[:, 1536:2048], in1=tch)
                # transpose h_out -> hT (128, 4, 8)
                tp = tps.tile([128, 4, BL], BF, tag="tp")
                for k in range(4):
                    nc.tensor.transpose(tp[:, k, :],
                                        h_out[:, k * 128 : (k + 1) * 128], i8)
                nc.vector.tensor_copy(out=hT, in_=tp)

            h0_out = lstm.tile([BL, H], BF, tag="h0o")
            for t in range(n_steps):
                g0t = lstm.tile([BL, G], BF, tag="g0t")
                nc.sync.dma_start(out=g0t, in_=g0_steps[t])
                # z0 = g0pre[t] + h0 @ Whh0s.T   (two column groups)
                z0 = zps.tile([128, 1024], F32, tag="zp")
                zb0 = [z0[p : p + BL, (j // 2) * 512 : (j // 2 + 1) * 512]
                       for j, p in enumerate(GP)]
                for k in range(4):
                    for j in range(4):
                        nc.tensor.matmul(
                            zb0[j], h0T[:, k, :], whh0T_s[:, k, GS[j]],
                            start=(k == 0), stop=False,
                            tile_position=(0, GP[j]),
                        )
                for j in range(4):
                    nc.tensor.matmul(zb0[j], i8, g0t[:, GS[j]],
                                     start=False, stop=True,
                                     tile_position=(0, GP[j]))
                cell(z0, h0T, c0, h0_out)

                # z1 = [h0', h1] @ W1cat.T + bias1
                z1 = zps.tile([128, 1024], F32, tag="zp")
                zb1 = [z1[p : p + BL, (j // 2) * 512 : (j // 2 + 1) * 512]
                       for j, p in enumerate(GP)]
                for k in range(4):
                    for j in range(4):
                        nc.tensor.matmul(
                            zb1[j], h0T[:, k, :], w1catT_s[:, k, GS[j]],
                            start=(k == 0), stop=False,
                            tile_position=(0, GP[j]),
                        )
                for k in range(4):
                    for j in range(4):
                        nc.tensor.matmul(
                            zb1[j], h1T[:, k, :], w1catT_s[:, 4 + k, GS[j]],
                            start=False, stop=False,
                            tile_position=(0, GP[j]),
                        )
                for j in range(4):
                    nc.tensor.matmul(zb1[j], ones_1x128[:, 0:BL],
                                     bias1r_s[:, GS[j]],
                                     start=False, stop=True,
                                     tile_position=(0, GP[j]))
                cell(z1, h1T, c1, h1_last)

            # ---------------- head ----------------
            def head_mm(lhs_tiles, wT_s_, bias_r, n):
                ps = zps.tile([128, n], F32, tag="zp")
                o = ps[0:BL, :]
                for k, lt in enumerate(lhs_tiles):
                    nc.tensor.matmul(o, lt, wT_s_[:, k, :],
                                     start=(k == 0), stop=False)
                nc.tensor.matmul(o, ones_1x128[:, 0:BL], bias_r,
                                 start=False, stop=True)
                return ps

            def layernorm_sigmoid(ps, dst):
                mv_s = small.tile([BL, 6], F32, tag="mv_s")
                nc.vector.bn_stats(out=mv_s, in_=ps[0:BL, :])
                mv = small.tile([BL, 2], F32, tag="mv")
                nc.vector.bn_aggr(out=mv, in_=mv_s)
                sd = small.tile([BL, 1], F32, tag="hsd")
                nc.scalar.activation(out=sd, in_=mv[:, 1:2], func=AF.Sqrt,
                                     bias=eps_col[0:BL, :])
                inv = small.tile([BL, 1], F32, tag="hinv")
                nc.vector.reciprocal(out=inv, in_=sd)
                tmp = small.tile([BL, H], F32, tag="lntmp")
                nc.vector.tensor_scalar(out=tmp, in0=ps[0:BL, :],
                                        scalar1=mv[:, 0:1], scalar2=inv,
                                        op0=ALU.subtract, op1=ALU.mult)
                nc.scalar.activation(out=dst, in_=tmp, func=AF.Sigmoid)

            def transpose_to(src_bf16, tag):
                tp = tps.tile([128, 4, BL], BF, tag="tp")
                for k in range(4):
                    nc.tensor.transpose(tp[:, k, :],
                                        src_bf16[:, k * 128 : (k + 1) * 128], i8)
                dst = small.tile([128, 4, BL], BF, tag=tag)
                nc.vector.tensor_copy(out=dst, in_=tp)
                return dst

            # comp gate
            cat_tiles = [h1T[:, k, :] for k in range(4)] + \
                        [ctxT_s[:, k, :] for k in range(4)]
            ps = head_mm(cat_tiles, wcT_s, bcr_s, H)
            comp = small.tile([BL, H], BF, tag="comp")
            layernorm_sigmoid(ps, comp)
            compressed = small.tile([BL, H], BF, tag="compr")
            nc.vector.tensor_mul(out=compressed, in0=h1_last, in1=comp)
            compT = transpose_to(compressed, "compT")

            # importance chain
            ps = head_mm([compT[:, k, :] for k in range(4)], wi1T_s, bi1r_s, H)
            i1 = small.tile([BL, H], BF, tag="i1")
            nc.scalar.activation(out=i1, in_=ps[0:BL, :], func=AF.Relu)
            i1T = transpose_to(i1, "i1T")
            ps = head_mm([i1T[:, k, :] for k in range(4)], wi2T_s, bi2r_s, H)
            imp = small.tile([BL, H], BF, tag="imp")
            layernorm_sigmoid(ps, imp)

            # si_new = si + rate * (mean|imp| - si)
            aimp = small.tile([BL, H], BF, tag="aimp")
            nc.scalar.activation(out=aimp, in_=imp, func=AF.Abs)
            sps_si = sps.tile([1, H], F32, tag="stat")
            nc.tensor.matmul(sps_si, ones_8x1, aimp, start=True, stop=True)
            sisum = small.tile([1, H], F32, tag="sisum")
            nc.vector.tensor_copy(out=sisum, in_=sps_si)
            cin = dram.tile([1, H], F32, tag="siin")
            cout = dram.tile([1, H], F32, tag="siout")
            nc.sync.dma_start(out=cin, in_=sisum)
            nc.gpsimd.collective_compute(
                "AllReduce", ALU.add, replica_groups=RG,
                ins=[cin.opt()], outs=[cout.opt()],
            )
            nc.sync.dma_start(out=sisum, in_=cout)
            nc.vector.tensor_scalar_mul(out=sisum, in0=sisum, scalar1=1.0 / B)
            nc.vector.tensor_sub(out=sisum, in0=sisum, in1=si_s)
            nc.vector.tensor_scalar_mul(out=sisum, in0=sisum,
                                        scalar1=ADAPTIVE_RATE)
            nc.vector.tensor_add(out=sisum, in0=sisum, in1=si_s)
            nc.sync.dma_start(out=si_out_d.ap(), in_=sisum)

            # final output
            wfT_s = wtile(wfT, H, OUT, "w4a")  # reuses wi1T slot
            ps = head_mm([compT[:, k, :] for k in range(4)], wfT_s, bfr_s, OUT)
            outf = small.tile([BL, OUT], F32, tag="outf")
            nc.vector.tensor_copy(out=outf, in_=ps[0:BL, :])
            nc.sync.dma_start(out=out_d.ap(), in_=outf)

    nc.compile()
    return nc


_CACHE = {}


def _get_compiled(n_steps):
    if n_steps not in _CACHE:
        nc = bacc.Bacc("TRN2", target_bir_lowering=False, num_devices=NCORES)
        _CACHE[n_steps] = build(nc, n_steps)
    return _CACHE[n_steps]


def _prep_inputs(inputs):
    f = lambda a: np.asarray(a, dtype=np.float32)
    bfc = lambda a: np.ascontiguousarray(a).astype(bf16)

    x = f(inputs["x"])
    ctx = f(inputs["context"])
    si = f(inputs["state_importance"]).reshape(1, H)

    wih0 = f(inputs["Wih0"])
    whh0 = f(inputs["Whh0"])
    bias0 = f(inputs["bih0"]) + f(inputs["bhh0"])
    wih1 = f(inputs["Wih1"])
    whh1 = f(inputs["Whh1"])
    bias1 = f(inputs["bih1"]) + f(inputs["bhh1"])

    colmaj = lambda b: np.ascontiguousarray(f(b).reshape(4, 128).T)

    shared = {
        "w_embT": bfc(f(inputs["W_emb"]).T),
        "b_embc": colmaj(inputs["b_emb"]),
        "w0T": bfc(f(inputs["W0"]).T),
        "b0c": colmaj(inputs["b0"]),
        "w1T": bfc(f(inputs["W1"]).T),
        "b1c": colmaj(inputs["b1"]),
        "wih0T": bfc(wih0.T),
        "bias0r": bfc(bias0.reshape(1, G)),
        "whh0T": bfc(whh0.T),
        "w1catT": bfc(np.concatenate([wih1.T, whh1.T], axis=0)),
        "bias1r": bfc(bias1.reshape(1, G)),
        "wcT": bfc(f(inputs["Wc"]).T),
        "bcr": bfc(f(inputs["bc"]).reshape(1, H)),
        "wi1T": bfc(f(inputs["Wi1"]).T),
        "bi1r": bfc(f(inputs["bi1"]).reshape(1, H)),
        "wi2T": bfc(f(inputs["Wi2"]).T),
        "bi2r": bfc(f(inputs["bi2"]).reshape(1, H)),
        "wfT": bfc(f(inputs["Wf"]).T),
        "bfr": bfc(f(inputs["bf"]).reshape(1, OUT)),
        "si_in": si,
    }

    in_maps = []
    for c in range(NCORES):
        xs = x[c * BL : (c + 1) * BL].reshape(NT, INDIM)
        m = dict(shared)
        m["xT"] = bfc(xs.T)
        m["ctxT"] = bfc(ctx[c * BL : (c + 1) * BL].T)
        in_maps.append(m)
    return in_maps


def kernel(trace=False, **inputs):
    nc = _get_compiled(N_STEPS)
    in_maps = _prep_inputs(inputs)
    res = run_bass_kernel_spmd(
        nc, in_maps, core_ids=list(range(NCORES)), trace=trace
    )
    out = np.concatenate([res.results[c]["out"] for c in range(NCORES)], axis=0)
    si_new = res.results[0]["si_new"]
    kernel.last_result = res
    return out, si_new


# revision 16
# speedup vs baseline: 1.2203x; 1.1266x over previous
# Trainium2 Bass kernel for nn_NeuralAdaptiveNetwork.
#
# Model: emb matmul -> 2x (Linear + BatchNorm1d(channel=S) + ReLU) ->
# 2-layer LSTMCell scan over S=512 steps -> gated head -> (out, si_new).
#
# Strategy: data-parallel over batch (B=64 -> 8 per core). Phase A (everything
# before the LSTM) runs in transposed-activation layout (feature on partitions,
# tokens on free dim, tokens b-major) so BatchNorm stats come out of PSUM
# accumulation for free; the two BN stat vectors and the final |imp| mean are
# the only cross-core collectives (AllReduce over 8 cores). The LSTM runs with
# batch on partitions, activations as the stationary matmul operand (weights
# stream through the PE), two concurrent column-group matmul streams via
# tile_position, and the precomputed input gates injected into PSUM with an
# 8x8 identity matmul. All matmuls in bf16, c-state and stats in fp32.

import os

import ml_dtypes
import numpy as np

import concourse.bacc as bacc
import concourse.tile as tile
from concourse import mybir
from concourse.bass_utils import run_bass_kernel_spmd
from concourse.masks import make_identity

BF = mybir.dt.bfloat16
F32 = mybir.dt.float32
AF = mybir.ActivationFunctionType
ALU = mybir.AluOpType

NCORES = 8
B, S, INDIM, H, OUT = 64, 512, 1024, 512, 512
BL = B // NCORES          # local batch = 8
NT = BL * S               # local tokens = 4096, b-major: t = b*S + s
G = 4 * H                 # 2048 gate dim
EPS = 1e-5
ADAPTIVE_RATE = 0.01
N_STEPS = int(os.environ.get("KERNEL_N_STEPS", S))

bf16 = ml_dtypes.bfloat16


def _bcast(ap, reps):
    """(P, F) AP -> (P, reps, F) AP with broadcast (step 0) middle dim."""
    p, f = ap.shape
    r = ap.rearrange("p (o f) -> p o f", o=1)
    return r.broadcast_to((p, reps, f))


def build(nc, n_steps):
    # ---------------- I/O ----------------
    def inp(name, shape, dt=BF):
        return nc.dram_tensor(name, shape, dt, kind="ExternalInput")

    xT = inp("xT", [INDIM, NT])
    w_embT = inp("w_embT", [INDIM, H])
    b_embc = inp("b_embc", [128, 4], F32)
    w0T = inp("w0T", [H, H])
    b0c = inp("b0c", [128, 4], F32)
    w1T = inp("w1T", [H, H])
    b1c = inp("b1c", [128, 4], F32)
    wih0T = inp("wih0T", [H, G])
    bias0r = inp("bias0r", [1, G])
    whh0T = inp("whh0T", [H, G])
    w1catT = inp("w1catT", [2 * H, G])
    bias1r = inp("bias1r", [1, G])
    wcT = inp("wcT", [2 * H, H])
    bcr = inp("bcr", [1, H])
    ctxT = inp("ctxT", [H, BL])
    wi1T = inp("wi1T", [H, H])
    bi1r = inp("bi1r", [1, H])
    wi2T = inp("wi2T", [H, H])
    bi2r = inp("bi2r", [1, H])
    wfT = inp("wfT", [H, OUT])
    bfr = inp("bfr", [1, OUT])
    si_in = inp("si_in", [1, H], F32)

    out_d = nc.dram_tensor("out", [BL, OUT], F32, kind="ExternalOutput")
    si_out_d = nc.dram_tensor("si_new", [1, H], F32, kind="ExternalOutput")

    RG = [list(range(NCORES))]
    INV_N = 1.0 / (B * H)  # BN stat normalizer (global batch x hidden)
    CH = 512               # phase-A token chunk == one local batch element
    NCH = NT // CH         # 8 chunks

    with tile.TileContext(nc) as tc:
        with (
            tc.tile_pool(name="const", bufs=1) as const,
            tc.tile_pool(name="wpool", bufs=1) as wpool,
            tc.tile_pool(name="acts", bufs=1) as acts,
            tc.tile_pool(name="mm_in", bufs=2) as mm_in,
            tc.tile_pool(name="small", bufs=1) as small,
            tc.tile_pool(name="lstm", bufs=2) as lstm,
            tc.tile_pool(name="zps", bufs=2, space="PSUM") as zps,
            tc.tile_pool(name="tps", bufs=2, space="PSUM") as tps,
            tc.tile_pool(name="sps", bufs=2, space="PSUM") as sps,
            tc.tile_pool(name="dram", bufs=1, space="DRAM") as dram,
            tc.tile_pool(name="state", bufs=1) as state,
        ):
            # ---------------- constants ----------------
            i8 = const.tile([8, 8], BF, tag="i8")
            make_identity(nc, i8)
            ones_1x128 = const.tile([1, 128], BF, tag="o1x128")
            nc.vector.memset(ones_1x128, 1.0)
            ones_1x128f = const.tile([1, 128], F32, tag="o1x128f")
            nc.vector.memset(ones_1x128f, 1.0)
            ones_128x1 = const.tile([128, 1], BF, tag="o128x1")
            nc.vector.memset(ones_128x1, 1.0)
            ones_8x1 = const.tile([8, 1], BF, tag="o8x1")
            nc.vector.memset(ones_8x1, 1.0)
            eps_col = const.tile([128, 1], F32, tag="eps")
            nc.vector.memset(eps_col, EPS)

            # ---------------- weight loading helpers ----------------
            def wtile(dr, rows, cols, tag):
                # (rows, cols) DRAM -> (128, rows//128, cols) SBUF
                nk = rows // 128
                t = wpool.tile([128, nk, cols], BF, tag=tag)
                nc.sync.dma_start(
                    out=t, in_=dr.ap().rearrange("(k p) c -> p k c", p=128)
                )
                return t

            def rtile(dr, shape, tag, dt=BF):
                t = wpool.tile(shape, dt, tag=tag)
                nc.sync.dma_start(out=t, in_=dr.ap())
                return t

            # phase-A weights (tags shared with later weights of same shape)
            w_embT_s = wtile(w_embT, INDIM, H, "w8")       # later: wcT
            w0T_s = wtile(w0T, H, H, "w4a")                # later: wi1T, wfT
            w1T_s = wtile(w1T, H, H, "w4b")                # later: wi2T
            wih0T_s = wtile(wih0T, H, G, "wg16")           # later: whh0T
            bias0r_s = rtile(bias0r, [1, G], "biasg")      # later: bias1r
            b_embc_s = rtile(b_embc, [128, 4], "b_embc", F32)
            b0c_s = rtile(b0c, [128, 4], "b0c", F32)
            b1c_s = rtile(b1c, [128, 4], "b1c", F32)
            w1catT_s = wtile(w1catT, 2 * H, G, "w1catT")   # resident all run
            si_s = small.tile([1, H], F32, tag="si_s")
            nc.sync.dma_start(out=si_s, in_=si_in.ap())
            ctxT_s = state.tile([128, 4, BL], BF, tag="ctxT")
            nc.sync.dma_start(
                out=ctxT_s, in_=ctxT.ap().rearrange("(k p) b -> p k b", p=128)
            )

            xT_r = xT.ap().rearrange("(k p) t -> p k t", p=128)

            h2_dram = dram.tile([H, NT], BF, tag="h2d")  # pre-BN L0 out
            h2_dram_r = h2_dram.rearrange("(k p) t -> p k t", p=128)
            g0_dram = dram.tile([NT, G], BF, tag="g0")
            l1buf = acts.tile([128, 4, NT], BF, tag="l1buf")

            def stats_psums():
                ssum = sps.tile([1, S], F32, tag="stat")
                ssq = sps.tile([1, S], F32, tag="stat")
                return ssum, ssq

            def stat_mms(ssum, ssq, src, first, last):
                # src: (128, CH) bf16 slice; accumulate col-sums over partitions
                nc.tensor.matmul(ssum, ones_128x1, src, start=first, stop=last)
                sq = mm_in.tile([128, CH], BF, tag="c4")
                nc.vector.tensor_mul(out=sq, in0=src, in1=src)
                nc.tensor.matmul(ssq, ones_128x1, sq, start=first, stop=last)

            def copy_bias(dst, ps, bias_col, flip):
                # psum -> sbuf bf16 with per-partition bias add
                if flip:
                    nc.vector.tensor_scalar_add(out=dst, in0=ps, scalar1=bias_col)
                else:
                    nc.scalar.activation(out=dst, in_=ps, func=AF.Identity,
                                         bias=bias_col, scale=1.0)

            # ---- fused emb + L0 (+ BN0 stats), chunk = one batch element ----
            ssum0, ssq0 = stats_psums()
            for c in range(NCH):
                cr = slice(c * CH, (c + 1) * CH)
                xc = mm_in.tile([128, 8, CH], BF, tag="xc")
                nc.sync.dma_start(out=xc, in_=xT_r[:, :, cr])
                embc = mm_in.tile([128, 4, CH], BF, tag="c4")
                for m in range(4):
                    ps = zps.tile([128, CH], F32, tag="zp")
                    for k in range(8):
                        nc.tensor.matmul(
                            ps, w_embT_s[:, k, m * 128 : (m + 1) * 128],
                            xc[:, k, :], start=(k == 0), stop=(k == 7),
                        )
                    copy_bias(embc[:, m, :], ps, b_embc_s[:, m : m + 1], m % 2)
                l0c = mm_in.tile([128, 4, CH], BF, tag="c4")
                for m in range(4):
                    ps = zps.tile([128, CH], F32, tag="zp")
                    for k in range(4):
                        nc.tensor.matmul(
                            ps, w0T_s[:, k, m * 128 : (m + 1) * 128],
                            embc[:, k, :], start=(k == 0), stop=(k == 3),
                        )
                    copy_bias(l0c[:, m, :], ps, b0c_s[:, m : m + 1], (m + 1) % 2)
                    stat_mms(ssum0, ssq0, l0c[:, m, :],
                             first=(c == 0 and m == 0), last=(c == 7 and m == 3))
                nc.sync.dma_start(out=h2_dram_r[:, :, cr], in_=l0c)

            # ---- BN stat finalize: AllReduce + per-channel inv/minv ----
            def bn_finalize(ssum, ssq):
                stats = small.tile([1, 2 * S], F32, tag="stats")
                nc.vector.tensor_copy(out=stats[:, 0:S], in_=ssum)
                nc.vector.tensor_copy(out=stats[:, S : 2 * S], in_=ssq)
                cin = dram.tile([1, 2 * S], F32, tag="ccin")
                cout = dram.tile([1, 2 * S], F32, tag="ccout")
                nc.sync.dma_start(out=cin, in_=stats)
                nc.gpsimd.collective_compute(
                    "AllReduce", ALU.add, replica_groups=RG,
                    ins=[cin.opt()], outs=[cout.opt()],
                )
                nc.sync.dma_start(out=stats, in_=cout)
                mean = small.tile([1, S], F32, tag="sc_m")
                nc.vector.tensor_scalar_mul(out=mean, in0=stats[:, 0:S],
                                            scalar1=INV_N)
                var = small.tile([1, S], F32, tag="sc_v")
                nc.vector.tensor_scalar_mul(out=var, in0=stats[:, S : 2 * S],
                                            scalar1=INV_N)
                tmp = small.tile([1, S], F32, tag="sc_t")
                nc.vector.tensor_mul(out=tmp, in0=mean, in1=mean)
                nc.vector.tensor_sub(out=var, in0=var, in1=tmp)
                nc.scalar.activation(out=tmp, in_=var, func=AF.Sqrt,
                                     bias=eps_col[0:1, :])
                inv = small.tile([1, S], F32, tag="sc_i")
                nc.vector.reciprocal(out=inv, in_=tmp)
                nc.vector.tensor_mul(out=tmp, in0=mean, in1=inv)  # tmp = m*inv
                bps = zps.tile([128, 2 * S], F32, tag="zp")
                nc.tensor.matmul(bps[:, 0:S], ones_1x128f, inv,
                                 start=True, stop=True)
                nc.tensor.matmul(bps[:, S : 2 * S], ones_1x128f, tmp,
                                 start=True, stop=True)
                inv_b = small.tile([128, S], BF, tag="inv_b")
                nc.vector.tensor_copy(out=inv_b, in_=bps[:, 0:S])
                minv_b = small.tile([128, S], BF, tag="minv_b")
                nc.vector.tensor_copy(out=minv_b, in_=bps[:, S : 2 * S])
                return inv_b, minv_b

            inv0, minv0 = bn_finalize(ssum0, ssq0)
            inv0_bc, minv0_bc = _bcast(inv0, 4), _bcast(minv0, 4)

            # ---- L1 (+ BN1 stats): stream h2 back, normalize, matmul ----
            ssum1, ssq1 = stats_psums()
            for c in range(NCH):
                cr = slice(c * CH, (c + 1) * CH)
                h2c = mm_in.tile([128, 4, CH], BF, tag="c4")
                nc.sync.dma_start(out=h2c, in_=h2_dram_r[:, :, cr])
                nc.vector.tensor_mul(out=h2c, in0=h2c, in1=inv0_bc)
                nc.vector.tensor_sub(out=h2c, in0=h2c, in1=minv0_bc)
                nc.vector.tensor_scalar_max(out=h2c, in0=h2c, scalar1=0.0)
                for m in range(4):
                    ps = zps.tile([128, CH], F32, tag="zp")
                    for k in range(4):
                        nc.tensor.matmul(
                            ps, w1T_s[:, k, m * 128 : (m + 1) * 128],
                            h2c[:, k, :], start=(k == 0), stop=(k == 3),
                        )
                    copy_bias(l1buf[:, m, cr], ps, b1c_s[:, m : m + 1], m % 2)
                    stat_mms(ssum1, ssq1, l1buf[:, m, cr],
                             first=(c == 0 and m == 0), last=(c == 7 and m == 3))

            inv1, minv1 = bn_finalize(ssum1, ssq1)
            inv1_bc, minv1_bc = _bcast(inv1, 4), _bcast(minv1, 4)

            # normalize l1buf in place -> h3
            for c in range(NCH):
                d = l1buf[:, :, c * CH : (c + 1) * CH]
                nc.vector.tensor_mul(out=d, in0=d, in1=inv1_bc)
                nc.vector.tensor_sub(out=d, in0=d, in1=minv1_bc)
                nc.vector.tensor_scalar_max(out=d, in0=d, scalar1=0.0)

            # ---- g0pre = h3 @ Wih0s.T + bias0s -> DRAM (NT, G) bf16 ----
            for mt in range(NT // 128):
                for half in range(2):
                    hr = slice(half * 1024, (half + 1) * 1024)
                    ps = zps.tile([128, 1024], F32, tag="zp")
                    for q in range(2):
                        qr = slice(q * 512, (q + 1) * 512)
                        gqr = slice(half * 1024 + q * 512,
                                    half * 1024 + (q + 1) * 512)
                        for k in range(4):
                            nc.tensor.matmul(
                                ps[:, qr], l1buf[:, k, mt * 128 : (mt + 1) * 128],
                                wih0T_s[:, k, gqr], start=(k == 0), stop=False,
                            )
                        nc.tensor.matmul(ps[:, qr], ones_1x128, bias0r_s[:, gqr],
                                         start=False, stop=True)
                    gsb = mm_in.tile([128, 1024], BF, tag="gsb")
                    if (mt + half) % 2 == 0:
                        nc.vector.tensor_copy(out=gsb, in_=ps)
                    else:
                        nc.scalar.copy(out=gsb, in_=ps)
                    nc.sync.dma_start(
                        out=g0_dram[mt * 128 : (mt + 1) * 128, hr], in_=gsb
                    )

            # ---- load LSTM/head weights into freed phase-A slots ----
            whh0T_s = wtile(whh0T, H, G, "wg16")
            bias1r_s = rtile(bias1r, [1, G], "biasg")
            wcT_s = wtile(wcT, 2 * H, H, "w8")
            wi1T_s = wtile(wi1T, H, H, "w4a")
            wi2T_s = wtile(wi2T, H, H, "w4b")
            bcr_s = rtile(bcr, [1, H], "bcr")
            bi1r_s = rtile(bi1r, [1, H], "bi1r")
            bi2r_s = rtile(bi2r, [1, H], "bi2r")
            bfr_s = rtile(bfr, [1, OUT], "bfr")

            # ---------------- LSTM over n_steps ----------------
            h0T = state.tile([128, 4, BL], BF, tag="h0T")
            h1T = state.tile([128, 4, BL], BF, tag="h1T")
            nc.vector.memset(h0T, 0.0)
            nc.vector.memset(h1T, 0.0)
            c0 = state.tile([BL, H], F32, tag="c0")
            c1 = state.tile([BL, H], F32, tag="c1")
            nc.vector.memset(c0, 0.0)
            nc.vector.memset(c1, 0.0)
            h1_last = state.tile([BL, H], BF, tag="h1_last")
            nc.vector.memset(h1_last, 0.0)

            g0_steps = g0_dram.rearrange("(b s) g -> s b g", b=BL)
            GP = [0, 32, 64, 96]                 # col-group base partitions
            GS = [slice(j * 512, (j + 1) * 512) for j in range(4)]  # gate blocks

            def cell(z, hT, cc, h_out):
                # z: psum (128, 1024); rows 0:8 = [i|f], rows 64:72 = [2g|o]
                sz = lstm.tile([BL, G], BF, tag="sz")
                nc.scalar.activation(out=sz[:, 0:512], in_=z[0:BL, 0:512],
                                     func=AF.Sigmoid)
                nc.scalar.activation(out=sz[:, 512:1024],
                                     in_=z[32 : 32 + BL, 0:512],
                                     func=AF.Sigmoid)
                nc.scalar.activation(out=sz[:, 1024:1536],
                                     in_=z[64 : 64 + BL, 512:1024], func=AF.Tanh)
                nc.scalar.activation(out=sz[:, 1536:2048],
                                     in_=z[96 : 96 + BL, 512:1024],
                                     func=AF.Sigmoid)
                jp = sps.tile([1, 512], F32, tag="stat")
                nc.tensor.matmul(jp, sz[:, 0:1], sz[:, 0:512],
                                 start=True, stop=True)
                ig = lstm.tile([BL, H], BF, tag="ig")
                nc.vector.tensor_mul(out=ig, in0=sz[:, 0:H], in1=sz[:, 1024:1536])
                nc.vector.tensor_mul(out=cc, in0=sz[:, H : 2 * H], in1=cc)
                nc.vector.tensor_add(out=cc, in0=cc, in1=ig)
                tch = lstm.tile([BL, H], BF, tag="tch")
                nc.scalar.activation(out=tch, in_=cc, func=AF.Tanh)
                nc.vector.tensor_mul(out=h_out, in0=sz[:, 1536:2048], in1=tch)
                # transpose h_out -> hT (128, 4, 8)
                tp = tps.tile([128, 4, BL], BF, tag="tp")
                for k in range(4):
                    nc.tensor.transpose(tp[:, k, :],
                                        h_out[:, k * 128 : (k + 1) * 128], i8)
                nc.vector.tensor_copy(out=hT, in_=tp)

            h0_out = lstm.tile([BL, H], BF, tag="h0o")
            for t in range(n_steps):
                g0t = lstm.tile([BL, G], BF, tag="g0t")
                nc.sync.dma_start(out=g0t, in_=g0_steps[t])
                # z0 = g0pre[t] + h0 @ Whh0s.T   (two column groups)
                z0 = zps.tile([128, 1024], F32, tag="zp")
                zb0 = [z0[p : p + BL, (j // 2) * 512 : (j // 2 + 1) * 512]
                       for j, p in enumerate(GP)]
                for k in range(4):
                    for j in range(4):
                        nc.tensor.matmul(
                            zb0[j], h0T[:, k, :], whh0T_s[:, k, GS[j]],
                            start=(k == 0), stop=False,
                            tile_position=(0, GP[j]),
                        )
                for j in range(4):
                    nc.tensor.matmul(zb0[j], i8, g0t[:, GS[j]],
                                     start=False, stop=True,
                                     tile_position=(0, GP[j]))
                cell(z0, h0T, c0, h0_out)

                # z1 = [h0', h1] @ W1cat.T + bias1
                z1 = zps.tile([128, 1024], F32, tag="zp")
                zb1 = [z1[p : p + BL, (j // 2) * 512 : (j // 2 + 1) * 512]
                       for j, p in enumerate(GP)]
                for k in range(4):
                    for j in range(4):
                        nc.tensor.matmul(
                            zb1[j], h1T[:, k, :], w1catT_s[:, 4 + k, GS[j]],
                            start=(k == 0), stop=False,
                            tile_position=(0, GP[j]),
                        )
                for k in range(4):
                    for j in range(4):
                        nc.tensor.matmul(
                            zb1[j], h0T[:, k, :], w1catT_s[:, k, GS[j]],
                            start=False, stop=False,
                            tile_position=(0, GP[j]),
                        )
                for j in range(4):
                    nc.tensor.matmul(zb1[j], ones_1x128[:, 0:BL],
                                     bias1r_s[:, GS[j]],
                                     start=False, stop=True,
                                     tile_position=(0, GP[j]))
                cell(z1, h1T, c1, h1_last)

            # ---------------- head ----------------
            def head_mm(lhs_tiles, wT_s_, bias_r, n):
                ps = zps.tile([128, n], F32, tag="zp")
                o = ps[0:BL, :]
                for k, lt in enumerate(lhs_tiles):
                    nc.tensor.matmul(o, lt, wT_s_[:, k, :],
                                     start=(k == 0), stop=False)
                nc.tensor.matmul(o, ones_1x128[:, 0:BL], bias_r,
                                 start=False, stop=True)
                return ps

            def layernorm_sigmoid(ps, dst):
                mv_s = small.tile([BL, 6], F32, tag="mv_s")
                nc.vector.bn_stats(out=mv_s, in_=ps[0:BL, :])
                mv = small.tile([BL, 2], F32, tag="mv")
                nc.vector.bn_aggr(out=mv, in_=mv_s)
                sd = small.tile([BL, 1], F32, tag="hsd")
                nc.scalar.activation(out=sd, in_=mv[:, 1:2], func=AF.Sqrt,
                                     bias=eps_col[0:BL, :])
                inv = small.tile([BL, 1], F32, tag="hinv")
                nc.vector.reciprocal(out=inv, in_=sd)
                tmp = small.tile([BL, H], F32, tag="lntmp")
                nc.vector.tensor_scalar(out=tmp, in0=ps[0:BL, :],
                                        scalar1=mv[:, 0:1], scalar2=inv,
                                        op0=ALU.subtract, op1=ALU.mult)
                nc.scalar.activation(out=dst, in_=tmp, func=AF.Sigmoid)

            def transpose_to(src_bf16, tag):
                tp = tps.tile([128, 4, BL], BF, tag="tp")
                for k in range(4):
                    nc.tensor.transpose(tp[:, k, :],
                                        src_bf16[:, k * 128 : (k + 1) * 128], i8)
                dst = small.tile([128, 4, BL], BF, tag=tag)
                nc.vector.tensor_copy(out=dst, in_=tp)
                return dst

            # comp gate
            cat_tiles = [h1T[:, k, :] for k in range(4)] + \
                        [ctxT_s[:, k, :] for k in range(4)]
            ps = head_mm(cat_tiles, wcT_s, bcr_s, H)
            comp = small.tile([BL, H], BF, tag="comp")
            layernorm_sigmoid(ps, comp)
            compressed = small.tile([BL, H], BF, tag="compr")
            nc.vector.tensor_mul(out=compressed, in0=h1_last, in1=comp)
            compT = transpose_to(compressed, "compT")

            # importance chain
            ps = head_mm([compT[:, k, :] for k in range(4)], wi1T_s, bi1r_s, H)
            i1 = small.tile([BL, H], BF, tag="i1")
            nc.scalar.activation(out=i1, in_=ps[0:BL, :], func=AF.Relu)
            i1T = transpose_to(i1, "i1T")
            ps = head_mm([i1T[:, k, :] for k in range(4)], wi2T_s, bi2r_s, H)
            imp = small.tile([BL, H], BF, tag="imp")
            layernorm_sigmoid(ps, imp)

            # si_new = si + rate * (mean|imp| - si)
            aimp = small.tile([BL, H], BF, tag="aimp")
            nc.scalar.activation(out=aimp, in_=imp, func=AF.Abs)
            sps_si = sps.tile([1, H], F32, tag="stat")
            nc.tensor.matmul(sps_si, ones_8x1, aimp, start=True, stop=True)
            sisum = small.tile([1, H], F32, tag="sisum")
            nc.vector.tensor_copy(out=sisum, in_=sps_si)
            cin = dram.tile([1, H], F32, tag="siin")
            cout = dram.tile([1, H], F32, tag="siout")
            nc.sync.dma_start(out=cin, in_=sisum)
            nc.gpsimd.collective_compute(
                "AllReduce", ALU.add, replica_groups=RG,
                ins=[cin.opt()], outs=[cout.opt()],
            )
            nc.sync.dma_start(out=sisum, in_=cout)
            nc.vector.tensor_scalar_mul(out=sisum, in0=sisum, scalar1=1.0 / B)
            nc.vector.tensor_sub(out=sisum, in0=sisum, in1=si_s)
            nc.vector.tensor_scalar_mul(out=sisum, in0=sisum,
                                        scalar1=ADAPTIVE_RATE)
            nc.vector.tensor_add(out=sisum, in0=sisum, in1=si_s)
            nc.sync.dma_start(out=si_out_d.ap(), in_=sisum)

            # final output
            wfT_s = wtile(wfT, H, OUT, "w4a")  # reuses wi1T slot
            ps = head_mm([compT[:, k, :] for k in range(4)], wfT_s, bfr_s, OUT)
            outf = small.tile([BL, OUT], F32, tag="outf")
            nc.vector.tensor_copy(out=outf, in_=ps[0:BL, :])
            nc.sync.dma_start(out=out_d.ap(), in_=outf)

    nc.compile()
    return nc


_CACHE = {}


def _get_compiled(n_steps):
    if n_steps not in _CACHE:
        nc = bacc.Bacc("TRN2", target_bir_lowering=False, num_devices=NCORES)
        _CACHE[n_steps] = build(nc, n_steps)
    return _CACHE[n_steps]


def _prep_inputs(inputs):
    f = lambda a: np.asarray(a, dtype=np.float32)
    bfc = lambda a: np.ascontiguousarray(a).astype(bf16)

    x = f(inputs["x"])
    ctx = f(inputs["context"])
    si = f(inputs["state_importance"]).reshape(1, H)

    wih0 = f(inputs["Wih0"])
    whh0 = f(inputs["Whh0"])
    bias0 = f(inputs["bih0"]) + f(inputs["bhh0"])
    wih1 = f(inputs["Wih1"])
    whh1 = f(inputs["Whh1"])
    bias1 = f(inputs["bih1"]) + f(inputs["bhh1"])

    colmaj = lambda b: np.ascontiguousarray(f(b).reshape(4, 128).T)

    shared = {
        "w_embT": bfc(f(inputs["W_emb"]).T),
        "b_embc": colmaj(inputs["b_emb"]),
        "w0T": bfc(f(inputs["W0"]).T),
        "b0c": colmaj(inputs["b0"]),
        "w1T": bfc(f(inputs["W1"]).T),
        "b1c": colmaj(inputs["b1"]),
        "wih0T": bfc(wih0.T),
        "bias0r": bfc(bias0.reshape(1, G)),
        "whh0T": bfc(whh0.T),
        "w1catT": bfc(np.concatenate([wih1.T, whh1.T], axis=0)),
        "bias1r": bfc(bias1.reshape(1, G)),
        "wcT": bfc(f(inputs["Wc"]).T),
        "bcr": bfc(f(inputs["bc"]).reshape(1, H)),
        "wi1T": bfc(f(inputs["Wi1"]).T),
        "bi1r": bfc(f(inputs["bi1"]).reshape(1, H)),
        "wi2T": bfc(f(inputs["Wi2"]).T),
        "bi2r": bfc(f(inputs["bi2"]).reshape(1, H)),
        "wfT": bfc(f(inputs["Wf"]).T),
        "bfr": bfc(f(inputs["bf"]).reshape(1, OUT)),
        "si_in": si,
    }

    in_maps = []
    for c in range(NCORES):
        xs = x[c * BL : (c + 1) * BL].reshape(NT, INDIM)
        m = dict(shared)
        m["xT"] = bfc(xs.T)
        m["ctxT"] = bfc(ctx[c * BL : (c + 1) * BL].T)
        in_maps.append(m)
    return in_maps


def kernel(trace=False, **inputs):
    nc = _get_compiled(N_STEPS)
    in_maps = _prep_inputs(inputs)
    res = run_bass_kernel_spmd(
        nc, in_maps, core_ids=list(range(NCORES)), trace=trace
    )
    out = np.concatenate([res.results[c]["out"] for c in range(NCORES)], axis=0)
    si_new = res.results[0]["si_new"]
    kernel.last_result = res
    return out, si_new
